# revision 19
# baseline (speedup 1.0000x reference)
"""DualAttention2d Trainium2 kernel.

Sharding: 8 cores = 4 samples x {spatial-attention branch, channel-attention
branch}. Core c < 4 computes the spatial branch of sample c; core c >= 4
computes the channel branch of sample c-4. Host sums the two branch outputs.

Single SPMD program; branch divergence via tc.If(partition_id < 4).

Layout notes:
- Feature maps on-chip as [4 blocks][128 chan, S] with S = 64*64 = 4096.
- Conv inputs live in a zero-padded [128, 66*66] buffer (1-px halo); a 3x3
  conv is 9 shifted matmuls accumulated in PSUM over 4 channel blocks.
- BN is folded into conv weights/bias on the host. alpha is folded into the
  v-projection, beta into the channel-attention softmax normalization.
- Matmuls run in float32r (full PE rate at N>=256, ~1e-4 rel precision).
  Attention probabilities and their transposes are bf16.
"""

import numpy as np

import concourse.bacc as bacc
import concourse.mybir as mybir
import concourse.tile as tile
from concourse.bass_utils import run_bass_kernel_spmd

B, C, H, W = 4, 512, 64, 64
S = H * W            # 4096
CI = 64              # q/k channels
P = 128
NB = C // P          # 4 channel blocks
PW = 66              # padded row width
PR = 66              # padded rows (1 zero row top/bottom)
PAD = PW * PR        # 4356
NST = S // 512       # 8 s-tiles of 512
EPS = 1e-5

F32 = mybir.dt.float32
F32R = mybir.dt.float32r
BF16 = mybir.dt.bfloat16
AF = mybir.ActivationFunctionType
AX = mybir.AxisListType

_CACHE = {}


def _pad_view(xpad_ap, st, dy=1, dx=1):
    """View of padded buffer [128, PAD] covering s-tile `st` (8 image rows x 64
    cols) shifted by tap (dy, dx) in {0,1,2}^2. dy=dx=1 is the centered view."""
    v = xpad_ap.rearrange("p (r w) -> p r w", w=PW)
    r0 = st * 8 + dy
    return v[:, r0:r0 + 8, dx:dx + 64]


def _conv(nc, tc, psA, psT, wpool, bounce, xpad_tiles, w_dram, b_dram,
          evict_fn, transpose_fn=None):
    """3x3 conv over the 4 padded input blocks; evict_fn(ob, st, psum) handles
    PSUM eviction. transpose_fn(ob, st, sb_tile) optionally consumes the
    evicted flat tile."""
    for ob in range(NB):
        for half in range(2):
            sts = range(half * 4, half * 4 + 4)
            psums = {st: psA.tile([P, 512], F32, tag="mm", name=f"cps{st}")
                     for st in sts}
            first, last = (0, 0), (8, NB - 1)
            for tap in range(9):
                dy, dx = tap // 3, tap % 3
                for cb in range(NB):
                    wt = wpool.tile([P, P], F32R, tag="w")
                    nc.sync.dma_start(wt[:], w_dram[tap, cb, ob])
                    for st in sts:
                        nc.tensor.matmul(
                            psums[st][:], wt[:],
                            _pad_view(xpad_tiles[cb][:], st, dy, dx),
                            start=((tap, cb) == first), stop=((tap, cb) == last))
            for st in sts:
                evict_fn(ob, st, psums[st])


def build():
    nc = bacc.Bacc("TRN2", target_bir_lowering=False, debug=False,
                   num_devices=8)

    # ---- I/O ----
    x_d = nc.dram_tensor("xpad", [NB, P, PAD], F32R, kind="ExternalInput")
    w1_d = nc.dram_tensor("w1", [9, NB, NB, P, P], F32R, kind="ExternalInput")
    b1_d = nc.dram_tensor("b1", [NB, P, 1], F32, kind="ExternalInput")
    w2_d = nc.dram_tensor("w2", [9, NB, NB, P, P], F32R, kind="ExternalInput")
    b2_d = nc.dram_tensor("b2", [NB, P, 1], F32, kind="ExternalInput")
    qw_d = nc.dram_tensor("qw", [NB, P, CI], F32R, kind="ExternalInput")
    kw_d = nc.dram_tensor("kw", [NB, P, CI], F32R, kind="ExternalInput")
    vw_d = nc.dram_tensor("vw", [NB, P, 512], F32R, kind="ExternalInput")
    qb_d = nc.dram_tensor("qb", [CI, 1], F32, kind="ExternalInput")
    kb_d = nc.dram_tensor("kb", [CI, 1], F32, kind="ExternalInput")
    vba_d = nc.dram_tensor("vba", [NB, P, 1], F32, kind="ExternalInput")
    beta_d = nc.dram_tensor("betat", [P, 1], F32, kind="ExternalInput")
    idr_d = nc.dram_tensor("identr", [P, P], F32R, kind="ExternalInput")
    idb_d = nc.dram_tensor("identb", [P, P], BF16, kind="ExternalInput")
    out_d = nc.dram_tensor("out", [NB, P, S], F32, kind="ExternalOutput")

    # ---- internal DRAM scratch ----
    s1_d = nc.dram_tensor("s1f", [NB, P, S], F32R, kind="Internal")
    c1t_d = nc.dram_tensor("c1t", [S // P, P, 512], F32R, kind="Internal")
    q_d = nc.dram_tensor("qs", [CI, S], F32R, kind="Internal")
    k_d = nc.dram_tensor("ks", [CI, S], F32R, kind="Internal")
    vt_d = nc.dram_tensor("vts", [S // P, P, 512], BF16, kind="Internal")

    with tile.TileContext(nc) as tc:
        ctx_pools = []

        def pool(name, bufs, space="SBUF"):
            p = tc.tile_pool(name=name, bufs=bufs, space=space)
            ctx_pools.append(p)
            return p.__enter__()

        psA = pool("psA", 6, space="PSUM")
        psT = pool("psT", 2, space="PSUM")
        xpadp = pool("xpadp", NB)
        wpool = pool("wpool", 12)
        bounce = pool("bounce", 4)
        b128 = pool("b128", 4)
        s1st = pool("s1st", 6)
        consts = pool("consts", 1)
        attTp = pool("attTp", S // P)
        kgp = pool("kgp", 1)
        qgp = pool("qgp", 2)
        logp = pool("logp", 1)
        probp = pool("probp", 1)
        cattp = pool("cattp", NB)
        vtbp = pool("vtbp", 2)
        statp = pool("statp", 8)

        # ---- constants ----
        ident_r = consts.tile([P, P], F32R, name="ident_r")
        nc.sync.dma_start(ident_r[:], idr_d.ap())
        ident_b = consts.tile([P, P], BF16, name="ident_b")
        nc.sync.dma_start(ident_b[:], idb_d.ap())
        qw_t = [consts.tile([P, CI], F32R, name=f"qw{i}") for i in range(NB)]
        kw_t = [consts.tile([P, CI], F32R, name=f"kw{i}") for i in range(NB)]
        vw_t = [consts.tile([P, 512], F32R, name=f"vw{i}") for i in range(NB)]
        b1_t = [consts.tile([P, 1], F32, name=f"b1{i}") for i in range(NB)]
        b2_t = [consts.tile([P, 1], F32, name=f"b2{i}") for i in range(NB)]
        vba_t = [consts.tile([P, 1], F32, name=f"vba{i}") for i in range(NB)]
        qb_t = consts.tile([CI, 1], F32, name="qbt")
        kb_t = consts.tile([CI, 1], F32, name="kbt")
        beta_t = consts.tile([P, 1], F32, name="betat_sb")
        for i in range(NB):
            nc.sync.dma_start(qw_t[i][:], qw_d[i])
            nc.sync.dma_start(kw_t[i][:], kw_d[i])
            nc.sync.dma_start(vw_t[i][:], vw_d[i])
            nc.sync.dma_start(b1_t[i][:], b1_d[i])
            nc.sync.dma_start(b2_t[i][:], b2_d[i])
            nc.sync.dma_start(vba_t[i][:], vba_d[i])
        nc.sync.dma_start(qb_t[:], qb_d.ap())
        nc.sync.dma_start(kb_t[:], kb_d.ap())
        nc.sync.dma_start(beta_t[:], beta_d.ap())

        # ---- load padded input ----
        xpad = [xpadp.tile([P, PAD], F32R, tag="xp", name=f"xpad{i}")
                for i in range(NB)]
        for i in range(NB):
            nc.sync.dma_start(xpad[i][:], x_d[i])

        # ---- conv1 (+ flat evict to DRAM, + transposed evict for c1T) ----
        def evict1(ob, st, psum):
            sb = bounce.tile([P, 512], F32R, tag="bn")
            nc.scalar.activation(sb[:], psum[:], AF.Relu, bias=b1_t[ob][:])
            nc.sync.dma_start(s1_d[ob, :, st * 512:(st + 1) * 512], sb[:])
            for j in range(4):
                pt = psT.tile([P, P], F32R, tag="tp")
                nc.tensor.transpose(pt[:], sb[:, j * P:(j + 1) * P],
                                    ident_r[:])
                tb = b128.tile([P, P], F32R, tag="t128")
                nc.scalar.activation(tb[:], pt[:], AF.Identity)
                nc.sync.dma_start(
                    c1t_d[st * 4 + j, :, ob * P:(ob + 1) * P], tb[:])

        _conv(nc, tc, psA, psT, wpool, bounce, xpad, w1_d, b1_t, evict1)

        pid = nc.partition_id()
        with tc.If(pid < 4) as cmp:
            # ======== spatial attention branch ========
            # phase 1: q, k, vT from s1
            for st in range(NST):
                s1t = [s1st.tile([P, 512], F32R, tag="s1s", name=f"s1t{i}")
                       for i in range(NB)]
                for cb in range(NB):
                    nc.sync.dma_start(
                        s1t[cb][:], s1_d[cb, :, st * 512:(st + 1) * 512])
                pq = psA.tile([CI, 512], F32, tag="mm")
                pk = psA.tile([CI, 512], F32, tag="mm")
                for cb in range(NB):
                    nc.tensor.matmul(pq[:], qw_t[cb][:], s1t[cb][:],
                                     start=(cb == 0), stop=(cb == NB - 1))
                for cb in range(NB):
                    nc.tensor.matmul(pk[:], kw_t[cb][:], s1t[cb][:],
                                     start=(cb == 0), stop=(cb == NB - 1))
                qb_sb = bounce.tile([CI, 512], F32R, tag="bn")
                nc.scalar.activation(qb_sb[:], pq[:], AF.Identity, bias=qb_t[:])
                nc.sync.dma_start(q_d.ap()[:, st * 512:(st + 1) * 512], qb_sb[:])
                kb_sb = bounce.tile([CI, 512], F32R, tag="bn")
                nc.scalar.activation(kb_sb[:], pk[:], AF.Identity, bias=kb_t[:])
                nc.sync.dma_start(k_d.ap()[:, st * 512:(st + 1) * 512], kb_sb[:])
                for j in range(4):
                    pv = psA.tile([P, 512], F32, tag="mm")
                    for cb in range(NB):
                        nc.tensor.matmul(
                            pv[:], s1t[cb][:, j * P:(j + 1) * P],
                            vw_t[cb][:],
                            start=(cb == 0), stop=(cb == NB - 1))
                    vtb = vtbp.tile([P, 512], BF16, tag="vtb")
                    nc.scalar.activation(vtb[:], pv[:], AF.Identity)
                    nc.sync.dma_start(vt_d[st * 4 + j], vtb[:])

            # phase 2: attention, one group of 512 query positions at a time
            for g in range(NST):
                kg = kgp.tile([CI, S], F32R, tag="kg")
                nc.sync.dma_start(kg[:], k_d.ap())
                qg = qgp.tile([CI, 512], F32R, tag="qg")
                nc.sync.dma_start(qg[:], q_d.ap()[:, g * 512:(g + 1) * 512])
                attT = [attTp.tile([P, 512], BF16, tag="attT",
                                   name=f"attT_{g}_{j}")
                        for j in range(S // P)]
                for blk in range(4):
                    logits = logp.tile([P, S], F32, tag="lg")
                    for st in range(NST):
                        pl = psA.tile([P, 512], F32, tag="mm")
                        nc.tensor.matmul(
                            pl[:], qg[:, blk * P:(blk + 1) * P],
                            kg[:, st * 512:(st + 1) * 512],
                            start=True, stop=True)
                        nc.scalar.activation(
                            logits[:, st * 512:(st + 1) * 512], pl[:],
                            AF.Identity)
                    negmax = statp.tile([P, 1], F32, tag="st")
                    nc.vector.reduce_max(negmax[:], logits[:], axis=AX.X,
                                         negate=True)
                    probs = probp.tile([P, S], BF16, tag="pb")
                    rowsum = statp.tile([P, 1], F32, tag="st")
                    nc.scalar.activation(probs[:], logits[:], AF.Exp,
                                         bias=negmax[:], accum_out=rowsum[:])
                    recip = statp.tile([P, 1], F32, tag="st")
                    nc.vector.reciprocal(recip[:], rowsum[:])
                    nc.vector.tensor_scalar_mul(probs[:], probs[:], recip[:])
                    for j in range(S // P):
                        pt = psT.tile([P, P], BF16, tag="tp")
                        nc.tensor.transpose(
                            pt[:], probs[:, j * P:(j + 1) * P], ident_b[:])
                        nc.scalar.activation(
                            attT[j][:, blk * P:(blk + 1) * P], pt[:],
                            AF.Identity)
                # o = vT^T @ attT, two channel blocks per pass; each streamed
                # vT chunk is consumed by both blocks of the pass immediately
                for half in range(2):
                    cbs = (2 * half, 2 * half + 1)
                    po = [psA.tile([P, 512], F32, tag="mm",
                                   name=f"po_{g}_{half}_{i}")
                          for i in range(2)]
                    for j in range(S // P):
                        vt = s1st.tile([P, 512], BF16, tag="s1s")
                        nc.sync.dma_start(vt[:], vt_d[j])
                        for i, cb in enumerate(cbs):
                            nc.tensor.matmul(
                                po[i][:], vt[:, cb * P:(cb + 1) * P],
                                attT[j][:], start=(j == 0),
                                stop=(j == S // P - 1))
                    for i, cb in enumerate(cbs):
                        ob_sb = bounce.tile([P, 512], F32, tag="bn")
                        nc.scalar.activation(ob_sb[:], po[i][:], AF.Identity,
                                             bias=vba_t[cb][:])
                        s1r = s1st.tile([P, 512], F32R, tag="s1s")
                        nc.sync.dma_start(
                            s1r[:], s1_d[cb, :, g * 512:(g + 1) * 512])
                        nc.vector.tensor_add(
                            _pad_view(xpad[cb][:], g), ob_sb[:], s1r[:])
        with cmp.Else():
            # ======== channel attention branch ========
            pg = [psA.tile([P, 512], F32, tag="mm", name=f"pg{cb}")
                  for cb in range(NB)]
            for j in range(S // P):
                c1t = s1st.tile([P, 512], F32R, tag="s1s")
                nc.sync.dma_start(c1t[:], c1t_d[j])
                for cb in range(NB):
                    nc.tensor.matmul(pg[cb][:],
                                     c1t[:, cb * P:(cb + 1) * P],
                                     c1t[:], start=(j == 0),
                                     stop=(j == S // P - 1))
            catt = []
            for cb in range(NB):
                negmax = statp.tile([P, 1], F32, tag="st")
                nc.vector.reduce_max(negmax[:], pg[cb][:], axis=AX.X,
                                     negate=True)
                ct = cattp.tile([P, 512], F32R, tag="ct", name=f"catt{cb}")
                rowsum = statp.tile([P, 1], F32, tag="st")
                nc.scalar.activation(ct[:], pg[cb][:], AF.Exp,
                                     bias=negmax[:], accum_out=rowsum[:])
                recip = statp.tile([P, 1], F32, tag="st")
                nc.vector.reciprocal(recip[:], rowsum[:])
                # fold beta into the normalization: catt = beta * softmax(G)
                nc.vector.tensor_mul(recip[:], recip[:], beta_t[:])
                ctn = cattp.tile([P, 512], F32R, tag="ctn", name=f"cattn{cb}")
                nc.scalar.activation(ctn[:], ct[:], AF.Identity,
                                     scale=recip[:])
                catt.append(ctn)
            for st in range(NST):
                c1s = [s1st.tile([P, 512], F32R, tag="s1s", name=f"c1s{i}")
                       for i in range(NB)]
                for cb in range(NB):
                    nc.sync.dma_start(
                        c1s[cb][:], s1_d[cb, :, st * 512:(st + 1) * 512])
                for kb in range(NB):
                    pc = psA.tile([P, 512], F32, tag="mm")
                    for cb in range(NB):
                        nc.tensor.matmul(
                            pc[:], catt[cb][:, kb * P:(kb + 1) * P],
                            c1s[cb][:], start=(cb == 0),
                            stop=(cb == NB - 1))
                    nc.vector.tensor_add(
                        _pad_view(xpad[kb][:], st), pc[:], c1s[kb][:])

        # ---- conv2 (shared) ----
        def evict2(ob, st, psum):
            sb = bounce.tile([P, 512], F32, tag="bn")
            nc.scalar.activation(sb[:], psum[:], AF.Relu, bias=b2_t[ob][:])
            nc.sync.dma_start(out_d[ob, :, st * 512:(st + 1) * 512], sb[:])

        _conv(nc, tc, psA, psT, wpool, bounce, xpad, w2_d, b2_t, evict2)

        for p in reversed(ctx_pools):
            p.__exit__(None, None, None)

    nc.compile()
    return nc


def _fold_conv(w, g, b, m, v):
    scale = g / np.sqrt(v + EPS)
    wf = (w * scale[:, None, None, None]).astype(np.float32)
    bf = (b - m * scale).astype(np.float32)
    # [O, CI, 3, 3] -> [tap, cb, ob, ci, o]
    wt = wf.transpose(2, 3, 1, 0).reshape(9, NB, P, NB, P).transpose(
        0, 1, 3, 2, 4)
    return np.ascontiguousarray(wt), bf.reshape(NB, P, 1)


def _pad_x(x):
    # x: [C, H, W] -> [NB, P, PAD]
    xp = np.zeros((NB, P, PR, PW), np.float32)
    xp[:, :, 1:65, 1:65] = x.reshape(NB, P, H, W)
    return xp.reshape(NB, P, PAD)


def prep_inputs(inputs):
    """Build the 8 per-core input maps from the full problem inputs."""
    x = np.asarray(inputs["x"], np.float32)
    alpha = float(np.asarray(inputs["alpha"]).reshape(-1)[0])
    beta = float(np.asarray(inputs["beta"]).reshape(-1)[0])

    w1s, b1s = _fold_conv(np.asarray(inputs["sa_w1"]), inputs["sa_g1"],
                          inputs["sa_b1"], inputs["sa_m1"], inputs["sa_v1"])
    w2s, b2s = _fold_conv(np.asarray(inputs["sa_w2"]), inputs["sa_g2"],
                          inputs["sa_b2"], inputs["sa_m2"], inputs["sa_v2"])
    w1c, b1c = _fold_conv(np.asarray(inputs["ca_w1"]), inputs["ca_g1"],
                          inputs["ca_b1"], inputs["ca_m1"], inputs["ca_v1"])
    w2c, b2c = _fold_conv(np.asarray(inputs["ca_w2"]), inputs["ca_g2"],
                          inputs["ca_b2"], inputs["ca_m2"], inputs["ca_v2"])

    qw = np.ascontiguousarray(np.asarray(inputs["q_w"], np.float32).T.reshape(
        NB, P, CI))
    kw = np.ascontiguousarray(np.asarray(inputs["k_w"], np.float32).T.reshape(
        NB, P, CI))
    vw = np.ascontiguousarray(
        (alpha * np.asarray(inputs["v_w"], np.float32)).T.reshape(NB, P, 512))
    qb = np.asarray(inputs["q_b"], np.float32).reshape(CI, 1)
    kb = np.asarray(inputs["k_b"], np.float32).reshape(CI, 1)
    vba = (alpha * np.asarray(inputs["v_b"], np.float32)).reshape(NB, P, 1)
    betat = np.full((P, 1), beta, np.float32)
    import ml_dtypes
    identr = np.eye(P, dtype=np.float32)
    identb = np.eye(P, dtype=ml_dtypes.bfloat16)

    zeros_qw = np.zeros_like(qw)
    zeros_vw = np.zeros_like(vw)
    zeros_b = np.zeros_like(qb)
    zeros_vba = np.zeros_like(vba)

    maps = []
    for core in range(8):
        b = core % 4
        xp = _pad_x(x[b])
        if core < 4:
            m = dict(xpad=xp, w1=w1s, b1=b1s, w2=w2s, b2=b2s,
                     qw=qw, kw=kw, vw=vw, qb=qb, kb=kb, vba=vba, betat=betat,
                     identr=identr, identb=identb)
        else:
            m = dict(xpad=xp, w1=w1c, b1=b1c, w2=w2c, b2=b2c,
                     qw=zeros_qw, kw=zeros_qw, vw=zeros_vw, qb=zeros_b,
                     kb=zeros_b, vba=zeros_vba, betat=betat,
                     identr=identr, identb=identb)
        maps.append(m)
    return maps


def kernel(**inputs):
    if "nc" not in _CACHE:
        _CACHE["nc"] = build()
    nc = _CACHE["nc"]
    maps = prep_inputs(inputs)
    res = run_bass_kernel_spmd(nc, maps, core_ids=list(range(8)))
    out = np.zeros((B, C, H, W), np.float32)
    for b in range(B):
        sa = res.results[b]["out"].reshape(C, H, W)
        ca = res.results[b + 4]["out"].reshape(C, H, W)
        out[b] = sa + ca
    return out
